# revision 6
# baseline (speedup 1.0000x reference)
"""Trainium2 Bass kernel for ActorNetworkOriginal (GNN message passing).

Strategy (8-core SPMD, data-parallel over destination nodes):
  - Host: add self-loops, compute GCN norm coefficients, sort edges by
    (owner core, 128-dst-node tile), pad each tile slot to a fixed chunk
    count, pack gather indices, precompute per-chunk one-hot scatter
    matrices (norm at [edge_lane, dst_local]) in bf16, and pre-gather
    the raw node features per edge (G streams) so layer 1 needs no
    device-side gather at all.
  - Device, per core (identical program, per-core data):
      Layer 1: per dst tile, accumulate S[k, dst] = sum_e X_ext[src_e,k]
      * norm_e with chunk matmuls (lhsT = pre-gathered G chunk [128,k],
      rhs = one-hot chunk), then h1_tile = W1'^T @ S + bias (+relu) --
      two small matmuls per tile, no dma_gather.  h1 @ W2 for the own
      rows is exchanged with a bf16 AllGather.  Layer 2 fetches per-edge
      messages in bf16 with dma_gather (by source node) and scatter-adds
      into each 128-dst-node tile with one-hot matmuls on the
      TensorEngine (PSUM f32).  The virtual-node net runs the same
      pipeline and overlaps the physical-net collective.  Graph
      mean-pool, summed skip connections, the current-virtual-node
      selection (one-hot matmul) and the 3-layer MLP run as before;
      each core emits logits for its 8 graphs.
"""

import numpy as np
import ml_dtypes

import concourse.bass as bass
import concourse.tile as tile
from concourse import bacc, mybir
from concourse.bass_utils import run_bass_kernel_spmd
from concourse.masks import make_identity

F32 = mybir.dt.float32
BF16 = mybir.dt.bfloat16
I16 = mybir.dt.int16
OP = mybir.AluOpType
AX = mybir.AxisListType
NPBF = ml_dtypes.bfloat16

B, NPG, NVG = 64, 500, 50          # graphs, phys/virt nodes per graph
DPF, DVF, D = 16, 8, 128           # feature dims
NC = 8                             # cores
NP, NV = B * NPG, B * NVG          # 32000, 3200 total nodes
NPC, NVC = NP // NC, NV // NC      # 4000, 400 own nodes per core
GPC = B // NC                      # 8 graphs per core
NPT = (NPC + 127) // 128           # 32 p dst tiles / core
NVT = (NVC + 127) // 128           # 4 v dst tiles / core
PPAD = NPT * 128                   # 4096
VPAD = NVT * 128                   # 512
GCH = 38                           # max chunks per dma_gather group
KP, KV = DPF + 1, DVF + 1          # ext feature dims (with bias row)

LAST_EXEC_NS = None
TRACE = False


# ----------------------------------------------------------------- host prep

def _prep_edges(edge_index, n_nodes, npc, ntiles):
    """Self-loops + norm; edges keyed by (core, dst tile); per-tile-slot
    padding to a core-independent chunk count.  Returns per-core flat
    arrays (src node id, dst_local, norm) and the chunk counts."""
    src = np.asarray(edge_index[0], dtype=np.int64)
    dst = np.asarray(edge_index[1], dtype=np.int64)
    loops = np.arange(n_nodes, dtype=np.int64)
    src = np.concatenate([src, loops])
    dst = np.concatenate([dst, loops])
    deg = np.bincount(dst, minlength=n_nodes).astype(np.float32)
    dis = 1.0 / np.sqrt(deg)
    norm = (dis[src] * dis[dst]).astype(np.float32)

    core = dst // npc
    rem = dst % npc
    tid = rem // 128
    dloc = rem % 128
    key = core * ntiles + tid
    order = np.argsort(key, kind="stable")
    src, dloc, norm, key = src[order], dloc[order], norm[order], key[order]
    counts = np.bincount(key, minlength=NC * ntiles).reshape(NC, ntiles)
    cpt = np.maximum(1, -(-counts.max(axis=0) // 128)).astype(int)
    csum = np.concatenate([[0], np.cumsum(counts.ravel())])
    epc = int(cpt.sum()) * 128
    src_p = np.zeros((NC, epc), np.int64)
    dl_p = np.zeros((NC, epc), np.int64)
    nm_p = np.zeros((NC, epc), np.float32)
    for c in range(NC):
        off = 0
        for t in range(ntiles):
            k = c * ntiles + t
            a, b = int(csum[k]), int(csum[k + 1])
            n = b - a
            src_p[c, off:off + n] = src[a:b]
            dl_p[c, off:off + n] = dloc[a:b]
            nm_p[c, off:off + n] = norm[a:b]
            off += int(cpt[t]) * 128
    return src_p, dl_p, nm_p, cpt


def _prep_edges_balanced(edge_index, n_nodes, npc, ntiles, tpad):
    """p-net variant: per-core degree-balanced node->tile assignment.
    Tiles get npc/ntiles nodes each, chosen so per-tile edge counts are
    near-equal.  Returns per-core flat arrays (src node id, dst slot
    within tile, graph of dst, norm), chunk counts, and the node
    permutation perm[c, t*128+pos] = global node id (-1 for gap)."""
    npt = npc // ntiles                    # nodes per tile (125)
    src = np.asarray(edge_index[0], dtype=np.int64)
    dst = np.asarray(edge_index[1], dtype=np.int64)
    loops = np.arange(n_nodes, dtype=np.int64)
    src = np.concatenate([src, loops])
    dst = np.concatenate([dst, loops])
    deg = np.bincount(dst, minlength=n_nodes).astype(np.float32)
    dis = 1.0 / np.sqrt(deg)
    norm = (dis[src] * dis[dst]).astype(np.float32)

    # balanced assignment per core
    tile_of = np.empty(n_nodes, np.int64)
    pos_of = np.empty(n_nodes, np.int64)
    for c in range(NC):
        lo = c * npc
        nodes = np.arange(lo, lo + npc)
        order = np.argsort(-deg[nodes], kind="stable")
        loads = np.zeros(ntiles)
        fill = np.zeros(ntiles, np.int64)
        for nd in nodes[order]:
            cand = np.where(fill < npt)[0]
            t = cand[np.argmin(loads[cand])]
            tile_of[nd] = t
            pos_of[nd] = fill[t]
            loads[t] += deg[nd]
            fill[t] += 1
    core = dst // npc
    tid = tile_of[dst]
    dloc = pos_of[dst]
    grph = dst // (n_nodes // B)           # graph id (0..B-1) -> local g below
    key = core * ntiles + tid
    order = np.argsort(key, kind="stable")
    src, dloc, norm, key, grph = (src[order], dloc[order], norm[order],
                                  key[order], grph[order])
    counts = np.bincount(key, minlength=NC * ntiles).reshape(NC, ntiles)
    cpt = np.maximum(1, -(-counts.max(axis=0) // 128)).astype(int)
    csum = np.concatenate([[0], np.cumsum(counts.ravel())])
    epc = int(cpt.sum()) * 128
    src_p = np.zeros((NC, epc), np.int64)
    dl_p = np.zeros((NC, epc), np.int64)
    nm_p = np.zeros((NC, epc), np.float32)
    gr_p = np.zeros((NC, epc), np.int64)
    for c in range(NC):
        off = 0
        for t in range(ntiles):
            k = c * ntiles + t
            a, b = int(csum[k]), int(csum[k + 1])
            n = b - a
            src_p[c, off:off + n] = src[a:b]
            dl_p[c, off:off + n] = dloc[a:b]
            nm_p[c, off:off + n] = norm[a:b]
            gr_p[c, off:off + n] = grph[a:b] % GPC
            off += int(cpt[t]) * 128
    # node permutation: perm[c, t*128+pos] = node id
    perm = np.full((NC, tpad), -1, np.int64)
    for nd in range(n_nodes):
        c = nd // npc
        perm[c, tile_of[nd] * 128 + pos_of[nd]] = nd
    return src_p, dl_p, nm_p, gr_p, cpt, perm


def _pack_idx(src):
    """[NC, E] node ids -> [NC, 128, E//16] int16 (16-partition wrap,
    replicated to all 8 partition groups)."""
    n = src.shape[1]
    w = src.astype(np.int16).reshape(NC, n // 16, 16).transpose(0, 2, 1)
    return np.ascontiguousarray(np.tile(w, (1, 8, 1)))


def _build_onehots(dl, nm, gr=None):
    """[NC, E] dst_local + norm (+ graph) -> [NC, 128, nchunk*W] bf16
    one-hot scatter matrices; W=128, or 136 with 8 graph-pool columns."""
    nch = dl.shape[1] // 128
    w = 128 if gr is None else 136
    oh = np.zeros((NC, 128, nch, w), np.float32)
    ci = np.broadcast_to(np.arange(NC)[:, None], dl.shape)
    ei = np.broadcast_to(np.arange(dl.shape[1])[None, :], dl.shape)
    oh[ci.ravel(), ei.ravel() % 128, ei.ravel() // 128, dl.ravel()] = nm.ravel()
    if gr is not None:
        valid = (nm != 0)
        oh[ci[valid], ei[valid] % 128, ei[valid] // 128,
           128 + gr[valid]] = nm[valid]
    return np.ascontiguousarray(oh.reshape(NC, 128, nch * w).astype(NPBF))


def _build_G(src, xTb):
    """[NC, E] src ids + [k, N] bf16 ext features -> [NC, 128, nch, k]
    pre-gathered per-edge feature chunks (lhsT layout)."""
    g = xTb[:, src]                              # [k, NC, E]
    g = np.transpose(g, (1, 2, 0))               # [NC, E, k]
    nch = g.shape[1] // 128
    k = g.shape[2]
    return np.ascontiguousarray(
        g.reshape(NC, nch, 128, k).transpose(0, 2, 1, 3))


def _groups(cpt, gch):
    out, start, acc = [], 0, 0
    for t, c in enumerate(cpt):
        if acc + c > gch and acc > 0:
            out.append((start, t))
            start, acc = t, 0
        acc += int(c)
    out.append((start, len(cpt)))
    return out


# ------------------------------------------------------------- device build

def _build(cpt_p, cpt_v):
    nc = bacc.Bacc("TRN2", target_bir_lowering=False, debug=False,
                   num_devices=NC, num_swdge_queues=4)

    chp = int(cpt_p.sum())          # total p chunks per core
    chv = int(cpt_v.sum())
    coff_p = np.concatenate([[0], np.cumsum(cpt_p)]).astype(int)
    coff_v = np.concatenate([[0], np.cumsum(cpt_v)]).astype(int)
    grp_p = _groups(cpt_p, GCH)
    grp_v = _groups(cpt_v, GCH)

    def din(name, shape, dtype=F32):
        return nc.dram_tensor(name, shape, dtype, kind="ExternalInput")

    # inputs
    pxT_o = din("p_xT_own", [KP, PPAD])
    vxT_o = din("v_xT_own", [KV, NVC])
    wf1p = din("wf1p", [KP, D], BF16)
    wep = din("wep", [KP, D])
    w2p = din("w2p", [D, D], BF16)
    b1p = din("b1p", [D, 1])
    b2p = din("b2p", [D, 1])
    wf1v = din("wf1v", [KV, D], BF16)
    wev = din("wev", [KV, D])
    w2v = din("w2v", [D, D], BF16)
    b1v = din("b1v", [D, 1])
    b2v = din("b2v", [D, 1])
    w1a = din("w1a", [D, D], BF16)
    w1b = din("w1b", [D, D], BF16)
    b1a = din("b1a", [D, 1])
    b1b = din("b1b", [D, 1])
    w2a = din("w2a", [D, D], BF16)
    w2b = din("w2b", [D, D], BF16)
    b2m = din("b2m", [D, 1])
    w3 = din("w3", [D, 1], BF16)
    b3 = din("b3", [1, 1])
    sel = din("sel", [128, NVT * GPC])
    gt = din("gt", [GPC, PPAD])
    gp_d = din("Gp", [128, chp, KP], BF16)
    gv_d = din("Gv", [128, chv, KV], BF16)
    idxp2 = din("idxs_p2", [128, chp * 8], I16)
    idxv1 = din("idxs_v1", [128, chv * 8], I16)
    ohp_d = din("oh_p", [128, chp * 136], BF16)
    ohv_d = din("oh_v", [128, chv * 128], BF16)

    out_d = nc.dram_tensor("out", [1, PPAD], F32, kind="ExternalOutput")

    # internal DRAM (bf16 exchange)
    ccip = nc.dram_tensor("ccip", [PPAD, D], BF16, kind="Internal")
    ccop = nc.dram_tensor("ccop", [NC * PPAD, D], BF16, kind="Internal",
                          addr_space="Shared")
    cciv = nc.dram_tensor("cciv", [NVC, D], BF16, kind="Internal")
    ccov = nc.dram_tensor("ccov", [NC * NVC, D], BF16, kind="Internal",
                          addr_space="Shared")

    with tile.TileContext(nc) as tc:
        with (
            tc.tile_pool(name="meta", bufs=1) as meta,
            tc.tile_pool(name="bigp", bufs=3) as bigp,
            tc.tile_pool(name="vsml", bufs=1) as vsml,
            tc.tile_pool(name="stp", bufs=3) as stp,
            tc.tile_pool(name="psA", bufs=3, space="PSUM") as psA,
            tc.tile_pool(name="psB", bufs=5, space="PSUM") as psB,
        ):
            def load(dram, tag=None):
                shape = list(dram.shape)
                t = meta.tile(shape, dram.dtype, tag=tag or dram.name)
                nc.sync.dma_start(out=t[:], in_=dram[:])
                return t

            # ---- constant / metadata loads
            gp_sb = load(gp_d)
            wf1p_sb = load(wf1p)
            b1p_sb = load(b1p)
            wep_sb = load(wep); w2p_sb = load(w2p)
            b2p_sb = load(b2p)
            gv_sb = load(gv_d)
            wf1v_sb = load(wf1v); wev_sb = load(wev); w2v_sb = load(w2v)
            b1v_sb = load(b1v); b2v_sb = load(b2v)
            w1a_sb = load(w1a); w1b_sb = load(w1b)
            b1a_sb = load(b1a); b1b_sb = load(b1b)
            w2a_sb = load(w2a); w2b_sb = load(w2b); b2m_sb = load(b2m)
            w3_sb = load(w3); b3_sb = load(b3); sel_sb = load(sel)
            gt_sb = load(gt)
            idxv1_sb = load(idxv1)
            idxp2_sb = load(idxp2)

            ident = meta.tile([128, 128], F32, tag="ident")
            make_identity(nc, ident[:])

            initp = meta.tile([128, PPAD], BF16, tag="initp")
            initv = meta.tile([128, VPAD], F32, tag="initv")
            nc.vector.memset(initv[:], 0.0)

            pxTo_sb = meta.tile([KP, PPAD], F32, tag="pxTo")
            nc.sync.dma_start(out=pxTo_sb[:], in_=pxT_o[:])
            vxTo_sb = meta.tile([KV, NVC], F32, tag="vxTo")
            nc.sync.dma_start(out=vxTo_sb[:], in_=vxT_o[:])

            # ---- initT own (feat x own nodes, tile layout)
            for j in range(PPAD // 512):
                acc = psB.tile([128, 512], F32, tag="b512", space="PSUM")
                nc.tensor.matmul(acc[:],
                                 wep_sb[:],
                                 pxTo_sb[:, j * 512:(j + 1) * 512],
                                 start=True, stop=True)
                nc.vector.tensor_copy(out=initp[:, j * 512:(j + 1) * 512],
                                      in_=acc[:])
            acc = psB.tile([128, 512], F32, tag="b512", space="PSUM")
            nc.tensor.matmul(acc[:, :NVC], wev_sb[:], vxTo_sb[:],
                             start=True, stop=True)
            nc.vector.tensor_copy(out=initv[:, :NVC], in_=acc[:, :NVC])

            with (
                tc.tile_pool(name="gat", bufs=2) as gat,
                tc.tile_pool(name="gatm", bufs=5) as gatm,
            ):
                def gcn_layer1(g_sb, kd, oh_dram, ohw, cpt, coff, groups,
                               h_sb, b_sb, wf_sb):
                    """Layer 1: no gather.  Per tile accumulate
                    S[k, dst] = sum_e G[e, k] * oh[e, dst], then
                    h1_tile = wf^T @ S + bias, relu."""
                    for (t0, t1) in groups:
                        gch = int(coff[t1] - coff[t0])
                        ohb = gat.tile([128, gch, ohw], BF16, tag="ohb")
                        nc.sync.dma_start(
                            out=ohb[:],
                            in_=oh_dram[:, int(coff[t0]) * ohw:
                                        int(coff[t1]) * ohw])
                        for t in range(t0, t1):
                            base = int(coff[t] - coff[t0])
                            n_ch = int(cpt[t])
                            s_ps = psA.tile([128, 136], F32, tag="pacc",
                                            space="PSUM")
                            for j in range(n_ch):
                                nc.tensor.matmul(
                                    s_ps[:kd, :128],
                                    g_sb[:, int(coff[t]) + j, :],
                                    ohb[:, base + j, :128],
                                    start=(j == 0), stop=(j == n_ch - 1))
                            s_sb = stp.tile([kd, 128], BF16,
                                            tag=f"ssb{kd}")
                            nc.vector.tensor_copy(out=s_sb[:],
                                                  in_=s_ps[:kd, :128])
                            acc = psA.tile([128, 136], F32, tag="pacc",
                                           space="PSUM")
                            nc.tensor.matmul(acc[:, :128], wf_sb[:], s_sb[:],
                                             start=True, stop=True)
                            h_ap = h_sb[:, t * 128:(t + 1) * 128]
                            nc.scalar.activation(
                                out=h_ap, in_=acc[:, :128],
                                func=mybir.ActivationFunctionType.Relu,
                                bias=b_sb[:, 0:1])

                def gcn_layer2(src_dram, idx_sb, oh_dram, ohw, cpt, coff,
                               groups, h_sb, b_sb, gsum=None, qoff=0):
                    """Layer 2: bf16 dma_gather of h1@W2 rows + one-hot
                    scatter matmuls."""
                    for gi, (t0, t1) in enumerate(groups):
                        gch = int(coff[t1] - coff[t0])
                        e0 = int(coff[t0]) * 128
                        m = gatm.tile([128, gch, D], BF16, tag="mbufb")
                        nc.gpsimd.dma_gather(
                            m[:], src_dram[:],
                            idx_sb[:, e0 // 16:(e0 + gch * 128) // 16],
                            gch * 128, gch * 128, D, single_packet=False,
                            queue_num=(qoff + gi) % 4)
                        ohb = gat.tile([128, gch, ohw], BF16, tag="ohb2")
                        nc.sync.dma_start(
                            out=ohb[:],
                            in_=oh_dram[:, int(coff[t0]) * ohw:
                                        int(coff[t1]) * ohw])
                        for t in range(t0, t1):
                            nw = ohw if gsum is not None else 128
                            acc = psA.tile([128, 136], F32, tag="pacc",
                                           space="PSUM")
                            base = int(coff[t] - coff[t0])
                            n_ch = int(cpt[t])
                            for j in range(n_ch):
                                nc.tensor.matmul(
                                    acc[:, :nw], m[:, base + j, :],
                                    ohb[:, base + j, :nw],
                                    start=(j == 0), stop=(j == n_ch - 1))
                            h_ap = h_sb[:, t * 128:(t + 1) * 128]
                            nc.scalar.activation(
                                out=h_ap,
                                in_=acc[:, :128],
                                func=mybir.ActivationFunctionType.Identity,
                                bias=b_sb[:, 0:1])
                            if gsum is not None:
                                nc.vector.tensor_tensor(
                                    out=gsum[:], in0=gsum[:],
                                    in1=acc[:, 128:136], op=OP.add)

                def xw2_own(h_sb, w_sb, cci, ntile, nrows):
                    off = 0
                    while off < ntile:
                        nb = min(4, ntile - off)
                        acc = psB.tile([128, nb * 128], F32, tag="b512",
                                       space="PSUM")
                        for j in range(nb):
                            i = off + j
                            nc.tensor.matmul(
                                acc[:, j * 128:(j + 1) * 128],
                                h_sb[:, i * 128:(i + 1) * 128],
                                w_sb[:], start=True, stop=True)
                        stg = stp.tile([128, nb, 128], BF16, tag="stg")
                        nc.vector.tensor_copy(out=stg[:], in_=acc[:])
                        nfull = min(nb, (nrows - off * 128) // 128)
                        if nfull > 0:
                            nc.sync.dma_start(
                                out=cci[off * 128:(off + nfull) * 128, :]
                                .rearrange("(t p) f -> p t f", p=128),
                                in_=stg[:, :nfull, :])
                        rem = nrows - (off + nfull) * 128
                        if 0 < rem < 128 and nfull < nb:
                            nc.sync.dma_start(
                                out=cci[(off + nfull) * 128:nrows, :],
                                in_=stg[:rem, nfull, :])
                        off += nb

                # ---- p layer 1 (no gather)
                h1p = bigp.tile([128, PPAD], BF16, tag="bigh")
                gcn_layer1(gp_sb, KP, ohp_d, 136, cpt_p, coff_p, grp_p,
                           h1p, b1p_sb, wf1p_sb)
                # ---- XW2p own rows + AllGather (bf16)
                xw2_own(h1p, w2p_sb, ccip, NPT, PPAD)
                nc.gpsimd.collective_compute(
                    "AllGather", OP.bypass,
                    ins=[ccip[:]], outs=[ccop[:]],
                    replica_groups=[list(range(NC))])

                # ---- v layer 1 (overlaps p AllGather)
                h1v = vsml.tile([128, VPAD], BF16, tag="h1v")
                gcn_layer1(gv_sb, KV, ohv_d, 128, cpt_v, coff_v, grp_v,
                           h1v, b1v_sb, wf1v_sb)
                xw2_own(h1v, w2v_sb, cciv, NVT, NVC)
                nc.gpsimd.collective_compute(
                    "AllGather", OP.bypass,
                    ins=[cciv[:]], outs=[ccov[:]],
                    replica_groups=[list(range(NC))])

                # ---- p layer 2 (with fused graph pooling); v layer 2 is
                # interleaved mid-chain so its pools finish under p's drain
                gsum = vsml.tile([128, GPC], F32, tag="gsum")
                nc.vector.memset(gsum[:], 0.0)
                h2p = bigp.tile([128, PPAD], BF16, tag="bigh")
                ph1 = grp_p[:len(grp_p) // 2]
                ph2 = grp_p[len(grp_p) // 2:]
                gcn_layer2(ccop, idxp2_sb, ohp_d, 136, cpt_p, coff_p,
                           ph1, h2p, b2p_sb, gsum=gsum)
                h2v = vsml.tile([128, VPAD], F32, tag="h2v")
                gcn_layer2(ccov, idxv1_sb, ohv_d, 128, cpt_v, coff_v,
                           grp_v, h2v, b2v_sb, qoff=len(ph1))
                gcn_layer2(ccop, idxp2_sb, ohp_d, 136, cpt_p, coff_p,
                           ph2, h2p, b2p_sb, gsum=gsum,
                           qoff=len(ph1) + len(grp_v))

                # ---- v pools / v summed / cur_v
                gv = vsml.tile([128, GPC], F32, tag="gv")
                for g in range(GPC):
                    nc.vector.reduce_sum(out=gv[:, g:g + 1],
                                         in_=h2v[:, g * NVG:(g + 1) * NVG],
                                         axis=AX.X)
                gvs = vsml.tile([128, GPC], F32, tag="gvs")
                nc.vector.tensor_scalar(out=gvs[:], in0=gv[:],
                                        scalar1=1.0 / NVG, scalar2=None,
                                        op0=OP.mult)
                vsum = vsml.tile([128, VPAD], F32, tag="vsum")
                nc.vector.tensor_tensor(out=vsum[:], in0=h2v[:], in1=initv[:],
                                        op=OP.add)
                for g in range(GPC):
                    nc.scalar.activation(
                        out=vsum[:, g * NVG:(g + 1) * NVG],
                        in_=vsum[:, g * NVG:(g + 1) * NVG],
                        func=mybir.ActivationFunctionType.Identity,
                        bias=gvs[:, g:g + 1])
                # cur_v = vsum rows selected by action, via transpose + matmul
                curv_ps = psA.tile([128, GPC], F32, tag="pacc", space="PSUM")
                for k in range(NVT):
                    trp = psA.tile([128, 128], F32, tag="pacc", space="PSUM")
                    nc.tensor.transpose(trp[:], vsum[:, k * 128:(k + 1) * 128],
                                        ident[:])
                    vs_rm = vsml.tile([128, 128], F32, tag="vsrm")
                    nc.vector.tensor_copy(out=vs_rm[:], in_=trp[:])
                    nc.tensor.matmul(curv_ps[:], vs_rm[:],
                                     sel_sb[:, k * GPC:(k + 1) * GPC],
                                     start=(k == 0), stop=(k == NVT - 1))
                # gc = gsum/NPG + b2p + curv   (graph mean of h2 + cur_v)
                gc = vsml.tile([128, GPC], F32, tag="gc")
                nc.vector.tensor_scalar(out=gc[:], in0=gsum[:],
                                        scalar1=1.0 / NPG,
                                        scalar2=b2p_sb[:, 0:1],
                                        op0=OP.mult, op1=OP.add)
                nc.vector.tensor_tensor(out=gc[:], in0=gc[:], in1=curv_ps[:],
                                        op=OP.add)
                # broadcast gc per graph across permuted columns via PE
                gcT_ps = psA.tile([128, 128], F32, tag="pacc", space="PSUM")
                nc.tensor.transpose(gcT_ps[:GPC, :],
                                    gc[:], ident[:])
                gcT = vsml.tile([GPC, 128], F32, tag="gcT")
                nc.vector.tensor_copy(out=gcT[:], in_=gcT_ps[:GPC, :])

                state = bigp.tile([128, PPAD], BF16, tag="bigh")
                nc.vector.tensor_tensor(out=state[:], in0=h2p[:],
                                        in1=initp[:], op=OP.add)
                for n in range(PPAD // 512):
                    sl = slice(n * 512, (n + 1) * 512)
                    gcx = psB.tile([128, 512], F32, tag="b512", space="PSUM")
                    nc.tensor.matmul(gcx[:], gcT[:], gt_sb[:, sl],
                                     start=True, stop=True)
                    nc.vector.tensor_tensor(out=state[:, sl],
                                            in0=state[:, sl],
                                            in1=gcx[:], op=OP.add)

                # ---- MLP
                mh1a = bigp.tile([128, PPAD], BF16, tag="bigh")
                mh1b = bigp.tile([128, PPAD], BF16, tag="bigh")
                for (w_sb, b_sb, mh) in ((w1a_sb, b1a_sb, mh1a),
                                         (w1b_sb, b1b_sb, mh1b)):
                    for n in range(PPAD // 512):
                        sl = slice(n * 512, (n + 1) * 512)
                        acc = psB.tile([128, 512], F32, tag="b512",
                                       space="PSUM")
                        nc.tensor.matmul(acc[:], w_sb[:], state[:, sl],
                                         start=True, stop=True)
                        nc.scalar.activation(
                            out=mh[:, sl], in_=acc[:],
                            func=mybir.ActivationFunctionType.Relu,
                            bias=b_sb[:, 0:1])
                mh2 = bigp.tile([128, PPAD], BF16, tag="bigh")
                for n in range(PPAD // 512):
                    sl = slice(n * 512, (n + 1) * 512)
                    acc = psB.tile([128, 512], F32, tag="b512", space="PSUM")
                    nc.tensor.matmul(acc[:], w2a_sb[:], mh1a[:, sl],
                                     start=True, stop=False)
                    nc.tensor.matmul(acc[:], w2b_sb[:], mh1b[:, sl],
                                     start=False, stop=True)
                    nc.scalar.activation(
                        out=mh2[:, sl], in_=acc[:],
                        func=mybir.ActivationFunctionType.Relu,
                        bias=b2m_sb[:, 0:1])
                for n in range(PPAD // 512):
                    sl = slice(n * 512, (n + 1) * 512)
                    accl = psA.tile([1, 512], F32, tag="pacc", space="PSUM")
                    nc.tensor.matmul(accl[:], w3_sb[:], mh2[:, sl],
                                     start=True, stop=True)
                    lgc = vsml.tile([1, 512], F32, tag="lgc")
                    nc.vector.tensor_scalar(
                        out=lgc[0:1, :], in0=accl[:], scalar1=b3_sb[0:1, 0:1],
                        scalar2=None, op0=OP.add)
                    nc.sync.dma_start(out=out_d[0:1, sl], in_=lgc[0:1, :])

    nc.compile()
    return nc


# ------------------------------------------------------------------- kernel

def kernel(**inputs):
    global LAST_EXEC_NS
    f = lambda k: np.asarray(inputs[k], dtype=np.float32)

    # edge preprocessing
    sp, dlp, nmp, grp_, cpt_p, perm = _prep_edges_balanced(
        np.asarray(inputs["p_edge_index"]), NP, NPC, NPT, PPAD)
    sv, dlv, nmv, cpt_v = _prep_edges(np.asarray(inputs["v_edge_index"]),
                                      NV, NVC, NVT)
    # L2 p rows live at permuted positions: node -> core*PPAD + tile*128+pos
    posmap = np.empty(NP, np.int64)          # node -> tile*128+pos
    for c in range(NC):
        valid = perm[c] >= 0
        posmap[perm[c][valid]] = np.nonzero(valid)[0]
    sp2 = (sp // NPC) * PPAD + posmap[sp]

    idxs_p2 = _pack_idx(sp2)
    idxs_v1 = _pack_idx(sv)
    oh_p = _build_onehots(dlp, nmp, grp_)
    oh_v = _build_onehots(dlv, nmv)

    # weights
    p_x = f("p_x"); v_x = f("v_x")
    wep = np.vstack([f("p_lin_w"), f("p_lin_b")[None, :]])
    wev = np.vstack([f("v_lin_w"), f("v_lin_b")[None, :]])
    wf1p = wep @ f("p_gcn_w1")
    wf1v = wev @ f("v_gcn_w1")
    pxT = np.vstack([p_x.T, np.ones((1, NP), np.float32)])
    vxT = np.vstack([v_x.T, np.ones((1, NV), np.float32)])
    act = np.asarray(inputs["high_level_action"]).astype(np.int64)

    # pre-gathered per-edge raw features (layer 1 needs no device gather)
    Gp = _build_G(sp, pxT.astype(NPBF))
    Gv = _build_G(sv, vxT.astype(NPBF))

    base = {
        "wf1p": wf1p.astype(NPBF), "wep": wep,
        "w2p": f("p_gcn_w2").astype(NPBF),
        "b1p": f("p_gcn_b1")[:, None], "b2p": f("p_gcn_b2")[:, None],
        "wf1v": wf1v.astype(NPBF), "wev": wev,
        "w2v": f("v_gcn_w2").astype(NPBF),
        "b1v": f("v_gcn_b1")[:, None], "b2v": f("v_gcn_b2")[:, None],
        "w1a": f("low_w1")[:, :D].astype(NPBF),
        "w1b": f("low_w1")[:, D:].astype(NPBF),
        "b1a": f("low_b1")[:D, None], "b1b": f("low_b1")[D:, None],
        "w2a": f("low_w2")[:D, :].astype(NPBF),
        "w2b": f("low_w2")[D:, :].astype(NPBF),
        "b2m": f("low_b2")[:, None],
        "w3": f("low_w3").astype(NPBF), "b3": f("low_b3")[:, None],
    }
    base = {k: (np.ascontiguousarray(v) if v.dtype == NPBF
                else np.ascontiguousarray(v, dtype=np.float32))
            for k, v in base.items()}

    pgraph = np.asarray(inputs["p_batch"]).astype(np.int64)

    in_maps = []
    for c in range(NC):
        selm = np.zeros((128, NVT * GPC), np.float32)
        for g in range(GPC):
            r = g * NVG + int(act[c * GPC + g])
            selm[r % 128, (r // 128) * GPC + g] = 1.0
        # permuted own features (gap columns zero) + graph one-hot
        pxo = np.zeros((KP, PPAD), np.float32)
        gtm = np.zeros((GPC, PPAD), np.float32)
        valid = perm[c] >= 0
        cols = np.nonzero(valid)[0]
        nodes = perm[c][valid]
        pxo[:, cols] = pxT[:, nodes]
        gtm[pgraph[nodes] % GPC, cols] = 1.0
        m = dict(base)
        m["p_xT_own"] = pxo
        m["v_xT_own"] = np.ascontiguousarray(
            vxT[:, c * NVC:(c + 1) * NVC])
        m["sel"] = selm
        m["gt"] = gtm
        m["idxs_p2"] = idxs_p2[c]
        m["idxs_v1"] = idxs_v1[c]
        m["oh_p"] = oh_p[c]; m["oh_v"] = oh_v[c]
        m["Gp"] = Gp[c]; m["Gv"] = Gv[c]
        in_maps.append(m)

    nc = _build(cpt_p, cpt_v)
    res = run_bass_kernel_spmd(nc, in_maps, core_ids=list(range(NC)),
                               trace=TRACE)
    LAST_EXEC_NS = res.exec_time_ns
    out = np.empty((NC, NPC), np.float32)
    for c in range(NC):
        lgv = res.results[c]["out"][0]
        valid = perm[c] >= 0
        out[c][perm[c][valid] - c * NPC] = lgv[valid]
    return out.reshape(B, NPG).astype(np.float32)


# revision 7
# speedup vs baseline: 1.0192x; 1.0192x over previous
"""Trainium2 Bass kernel for ActorNetworkOriginal (GNN message passing).

Strategy (8-core SPMD, data-parallel over destination nodes):
  - Host: add self-loops, compute GCN norm coefficients, sort edges by
    (owner core, 128-dst-node tile), pad each tile slot to a fixed chunk
    count, pack gather indices, precompute per-chunk one-hot scatter
    matrices (norm at [edge_lane, dst_local]) in bf16, and pre-gather
    the raw node features per edge (G streams) so layer 1 needs no
    device-side gather at all.
  - Device, per core (identical program, per-core data):
      Layer 1: per dst tile, accumulate S[k, dst] = sum_e X_ext[src_e,k]
      * norm_e with chunk matmuls (lhsT = pre-gathered G chunk [128,k],
      rhs = one-hot chunk), then h1_tile = W1'^T @ S + bias (+relu) --
      two small matmuls per tile, no dma_gather.  h1 @ W2 for the own
      rows is exchanged with a bf16 AllGather.  Layer 2 fetches per-edge
      messages in bf16 with dma_gather (by source node) and scatter-adds
      into each 128-dst-node tile with one-hot matmuls on the
      TensorEngine (PSUM f32).  The virtual-node net runs the same
      pipeline and overlaps the physical-net collective.  Graph
      mean-pool, summed skip connections, the current-virtual-node
      selection (one-hot matmul) and the 3-layer MLP run as before;
      each core emits logits for its 8 graphs.
"""

import numpy as np
import ml_dtypes

import concourse.bass as bass
import concourse.tile as tile
from concourse import bacc, mybir
from concourse.bass_utils import run_bass_kernel_spmd
from concourse.masks import make_identity

F32 = mybir.dt.float32
BF16 = mybir.dt.bfloat16
I16 = mybir.dt.int16
OP = mybir.AluOpType
AX = mybir.AxisListType
NPBF = ml_dtypes.bfloat16

B, NPG, NVG = 64, 500, 50          # graphs, phys/virt nodes per graph
DPF, DVF, D = 16, 8, 128           # feature dims
NC = 8                             # cores
NP, NV = B * NPG, B * NVG          # 32000, 3200 total nodes
NPC, NVC = NP // NC, NV // NC      # 4000, 400 own nodes per core
GPC = B // NC                      # 8 graphs per core
NPT = (NPC + 127) // 128           # 32 p dst tiles / core
NVT = (NVC + 127) // 128           # 4 v dst tiles / core
PPAD = NPT * 128                   # 4096
VPAD = NVT * 128                   # 512
GCH = 38                           # max chunks per dma_gather group
KP, KV = DPF + 1, DVF + 1          # ext feature dims (with bias row)

LAST_EXEC_NS = None
TRACE = False


# ----------------------------------------------------------------- host prep

def _prep_edges(edge_index, n_nodes, npc, ntiles):
    """Self-loops + norm; edges keyed by (core, dst tile); per-tile-slot
    padding to a core-independent chunk count.  Returns per-core flat
    arrays (src node id, dst_local, norm) and the chunk counts."""
    src = np.asarray(edge_index[0], dtype=np.int64)
    dst = np.asarray(edge_index[1], dtype=np.int64)
    loops = np.arange(n_nodes, dtype=np.int64)
    src = np.concatenate([src, loops])
    dst = np.concatenate([dst, loops])
    deg = np.bincount(dst, minlength=n_nodes).astype(np.float32)
    dis = 1.0 / np.sqrt(deg)
    norm = (dis[src] * dis[dst]).astype(np.float32)

    core = dst // npc
    rem = dst % npc
    tid = rem // 128
    dloc = rem % 128
    key = core * ntiles + tid
    order = np.argsort(key, kind="stable")
    src, dloc, norm, key = src[order], dloc[order], norm[order], key[order]
    counts = np.bincount(key, minlength=NC * ntiles).reshape(NC, ntiles)
    cpt = np.maximum(1, -(-counts.max(axis=0) // 128)).astype(int)
    csum = np.concatenate([[0], np.cumsum(counts.ravel())])
    epc = int(cpt.sum()) * 128
    src_p = np.zeros((NC, epc), np.int64)
    dl_p = np.zeros((NC, epc), np.int64)
    nm_p = np.zeros((NC, epc), np.float32)
    for c in range(NC):
        off = 0
        for t in range(ntiles):
            k = c * ntiles + t
            a, b = int(csum[k]), int(csum[k + 1])
            n = b - a
            src_p[c, off:off + n] = src[a:b]
            dl_p[c, off:off + n] = dloc[a:b]
            nm_p[c, off:off + n] = norm[a:b]
            off += int(cpt[t]) * 128
    return src_p, dl_p, nm_p, cpt


def _prep_edges_balanced(edge_index, n_nodes, npc, ntiles, tpad):
    """p-net variant: per-core degree-balanced node->tile assignment.
    Tiles get npc/ntiles nodes each, chosen so per-tile edge counts are
    near-equal.  Returns per-core flat arrays (src node id, dst slot
    within tile, graph of dst, norm), chunk counts, and the node
    permutation perm[c, t*128+pos] = global node id (-1 for gap)."""
    npt = npc // ntiles                    # nodes per tile (125)
    src = np.asarray(edge_index[0], dtype=np.int64)
    dst = np.asarray(edge_index[1], dtype=np.int64)
    loops = np.arange(n_nodes, dtype=np.int64)
    src = np.concatenate([src, loops])
    dst = np.concatenate([dst, loops])
    deg = np.bincount(dst, minlength=n_nodes).astype(np.float32)
    dis = 1.0 / np.sqrt(deg)
    norm = (dis[src] * dis[dst]).astype(np.float32)

    # balanced assignment per core
    tile_of = np.empty(n_nodes, np.int64)
    pos_of = np.empty(n_nodes, np.int64)
    for c in range(NC):
        lo = c * npc
        nodes = np.arange(lo, lo + npc)
        order = np.argsort(-deg[nodes], kind="stable")
        loads = np.zeros(ntiles)
        fill = np.zeros(ntiles, np.int64)
        for nd in nodes[order]:
            cand = np.where(fill < npt)[0]
            t = cand[np.argmin(loads[cand])]
            tile_of[nd] = t
            pos_of[nd] = fill[t]
            loads[t] += deg[nd]
            fill[t] += 1
    core = dst // npc
    tid = tile_of[dst]
    dloc = pos_of[dst]
    grph = dst // (n_nodes // B)           # graph id (0..B-1) -> local g below
    key = core * ntiles + tid
    order = np.argsort(key, kind="stable")
    src, dloc, norm, key, grph = (src[order], dloc[order], norm[order],
                                  key[order], grph[order])
    counts = np.bincount(key, minlength=NC * ntiles).reshape(NC, ntiles)
    cpt = np.maximum(1, -(-counts.max(axis=0) // 128)).astype(int)
    csum = np.concatenate([[0], np.cumsum(counts.ravel())])
    epc = int(cpt.sum()) * 128
    src_p = np.zeros((NC, epc), np.int64)
    dl_p = np.zeros((NC, epc), np.int64)
    nm_p = np.zeros((NC, epc), np.float32)
    gr_p = np.zeros((NC, epc), np.int64)
    for c in range(NC):
        off = 0
        for t in range(ntiles):
            k = c * ntiles + t
            a, b = int(csum[k]), int(csum[k + 1])
            n = b - a
            src_p[c, off:off + n] = src[a:b]
            dl_p[c, off:off + n] = dloc[a:b]
            nm_p[c, off:off + n] = norm[a:b]
            gr_p[c, off:off + n] = grph[a:b] % GPC
            off += int(cpt[t]) * 128
    # node permutation: perm[c, t*128+pos] = node id
    perm = np.full((NC, tpad), -1, np.int64)
    for nd in range(n_nodes):
        c = nd // npc
        perm[c, tile_of[nd] * 128 + pos_of[nd]] = nd
    return src_p, dl_p, nm_p, gr_p, cpt, perm


def _pack_idx(src):
    """[NC, E] node ids -> [NC, 128, E//16] int16 (16-partition wrap,
    replicated to all 8 partition groups)."""
    n = src.shape[1]
    w = src.astype(np.int16).reshape(NC, n // 16, 16).transpose(0, 2, 1)
    return np.ascontiguousarray(np.tile(w, (1, 8, 1)))


def _build_onehots(dl, nm, gr=None):
    """[NC, E] dst_local + norm (+ graph) -> [NC, 128, nchunk*W] bf16
    one-hot scatter matrices; W=128, or 136 with 8 graph-pool columns."""
    nch = dl.shape[1] // 128
    w = 128 if gr is None else 136
    oh = np.zeros((NC, 128, nch, w), np.float32)
    ci = np.broadcast_to(np.arange(NC)[:, None], dl.shape)
    ei = np.broadcast_to(np.arange(dl.shape[1])[None, :], dl.shape)
    oh[ci.ravel(), ei.ravel() % 128, ei.ravel() // 128, dl.ravel()] = nm.ravel()
    if gr is not None:
        valid = (nm != 0)
        oh[ci[valid], ei[valid] % 128, ei[valid] // 128,
           128 + gr[valid]] = nm[valid]
    return np.ascontiguousarray(oh.reshape(NC, 128, nch * w).astype(NPBF))


def _build_G(src, xTb):
    """[NC, E] src ids + [k, N] bf16 ext features -> [NC, 128, nch, k]
    pre-gathered per-edge feature chunks (lhsT layout)."""
    g = xTb[:, src]                              # [k, NC, E]
    g = np.transpose(g, (1, 2, 0))               # [NC, E, k]
    nch = g.shape[1] // 128
    k = g.shape[2]
    return np.ascontiguousarray(
        g.reshape(NC, nch, 128, k).transpose(0, 2, 1, 3))


def _groups(cpt, gch):
    out, start, acc = [], 0, 0
    for t, c in enumerate(cpt):
        if acc + c > gch and acc > 0:
            out.append((start, t))
            start, acc = t, 0
        acc += int(c)
    out.append((start, len(cpt)))
    return out


# ------------------------------------------------------------- device build

def _build(cpt_p, cpt_v):
    nc = bacc.Bacc("TRN2", target_bir_lowering=False, debug=False,
                   num_devices=NC, num_swdge_queues=4)

    chp = int(cpt_p.sum())          # total p chunks per core
    chv = int(cpt_v.sum())
    coff_p = np.concatenate([[0], np.cumsum(cpt_p)]).astype(int)
    coff_v = np.concatenate([[0], np.cumsum(cpt_v)]).astype(int)
    grp_p = _groups(cpt_p, GCH)
    grp_v = _groups(cpt_v, GCH)

    def din(name, shape, dtype=F32):
        return nc.dram_tensor(name, shape, dtype, kind="ExternalInput")

    # inputs
    pxT_o = din("p_xT_own", [KP, PPAD])
    vxT_o = din("v_xT_own", [KV, NVC])
    wf1p = din("wf1p", [KP, D], BF16)
    wep = din("wep", [KP, D])
    w2p = din("w2p", [D, D], BF16)
    b1p = din("b1p", [D, 1])
    b2p = din("b2p", [D, 1])
    wf1v = din("wf1v", [KV, D], BF16)
    wev = din("wev", [KV, D])
    w2v = din("w2v", [D, D], BF16)
    b1v = din("b1v", [D, 1])
    b2v = din("b2v", [D, 1])
    w1a = din("w1a", [D, D], BF16)
    w1b = din("w1b", [D, D], BF16)
    b1a = din("b1a", [D, 1])
    b1b = din("b1b", [D, 1])
    w2a = din("w2a", [D, D], BF16)
    w2b = din("w2b", [D, D], BF16)
    b2m = din("b2m", [D, 1])
    w3 = din("w3", [D, 1], BF16)
    b3 = din("b3", [1, 1])
    sel = din("sel", [128, NVT * GPC])
    gt = din("gt", [GPC, PPAD])
    gp_d = din("Gp", [128, chp, KP], BF16)
    gv_d = din("Gv", [128, chv, KV], BF16)
    idxp2 = din("idxs_p2", [128, chp * 8], I16)
    idxv1 = din("idxs_v1", [128, chv * 8], I16)
    ohp_d = din("oh_p", [128, chp * 136], BF16)
    ohv_d = din("oh_v", [128, chv * 128], BF16)

    out_d = nc.dram_tensor("out", [1, PPAD], F32, kind="ExternalOutput")

    # internal DRAM (bf16 exchange)
    ccip = nc.dram_tensor("ccip", [PPAD, D], BF16, kind="Internal")
    ccop = nc.dram_tensor("ccop", [NC * PPAD, D], BF16, kind="Internal",
                          addr_space="Shared")
    cciv = nc.dram_tensor("cciv", [NVC, D], BF16, kind="Internal")
    ccov = nc.dram_tensor("ccov", [NC * NVC, D], BF16, kind="Internal",
                          addr_space="Shared")

    with tile.TileContext(nc) as tc:
        with (
            tc.tile_pool(name="meta", bufs=1) as meta,
            tc.tile_pool(name="bigp", bufs=3) as bigp,
            tc.tile_pool(name="vsml", bufs=1) as vsml,
            tc.tile_pool(name="stp", bufs=3) as stp,
            tc.tile_pool(name="psA", bufs=3, space="PSUM") as psA,
            tc.tile_pool(name="psB", bufs=5, space="PSUM") as psB,
        ):
            def load(dram, tag=None):
                shape = list(dram.shape)
                t = meta.tile(shape, dram.dtype, tag=tag or dram.name)
                nc.sync.dma_start(out=t[:], in_=dram[:])
                return t

            # ---- constant / metadata loads
            gp_sb = load(gp_d)
            wf1p_sb = load(wf1p)
            b1p_sb = load(b1p)
            wep_sb = load(wep); w2p_sb = load(w2p)
            b2p_sb = load(b2p)
            gv_sb = load(gv_d)
            wf1v_sb = load(wf1v); wev_sb = load(wev); w2v_sb = load(w2v)
            b1v_sb = load(b1v); b2v_sb = load(b2v)
            w1a_sb = load(w1a); w1b_sb = load(w1b)
            b1a_sb = load(b1a); b1b_sb = load(b1b)
            w2a_sb = load(w2a); w2b_sb = load(w2b); b2m_sb = load(b2m)
            w3_sb = load(w3); b3_sb = load(b3); sel_sb = load(sel)
            gt_sb = load(gt)
            idxv1_sb = load(idxv1)
            idxp2_sb = load(idxp2)

            ident = meta.tile([128, 128], F32, tag="ident")
            make_identity(nc, ident[:])

            initp = meta.tile([128, PPAD], BF16, tag="initp")
            initv = meta.tile([128, VPAD], F32, tag="initv")
            nc.vector.memset(initv[:], 0.0)

            pxTo_sb = meta.tile([KP, PPAD], F32, tag="pxTo")
            nc.sync.dma_start(out=pxTo_sb[:], in_=pxT_o[:])
            vxTo_sb = meta.tile([KV, NVC], F32, tag="vxTo")
            nc.sync.dma_start(out=vxTo_sb[:], in_=vxT_o[:])

            # ---- initT own (feat x own nodes, tile layout)
            for j in range(PPAD // 512):
                acc = psB.tile([128, 512], F32, tag="b512", space="PSUM")
                nc.tensor.matmul(acc[:],
                                 wep_sb[:],
                                 pxTo_sb[:, j * 512:(j + 1) * 512],
                                 start=True, stop=True)
                nc.vector.tensor_copy(out=initp[:, j * 512:(j + 1) * 512],
                                      in_=acc[:])
            acc = psB.tile([128, 512], F32, tag="b512", space="PSUM")
            nc.tensor.matmul(acc[:, :NVC], wev_sb[:], vxTo_sb[:],
                             start=True, stop=True)
            nc.vector.tensor_copy(out=initv[:, :NVC], in_=acc[:, :NVC])

            with (
                tc.tile_pool(name="gat", bufs=2) as gat,
                tc.tile_pool(name="gatm", bufs=6) as gatm,
            ):
                def gcn_layer1(g_sb, kd, oh_dram, ohw, cpt, coff, groups,
                               h_sb, b_sb, wf_sb):
                    """Layer 1: no gather.  Per tile accumulate
                    S[k, dst] = sum_e G[e, k] * oh[e, dst], then
                    h1_tile = wf^T @ S + bias, relu."""
                    for (t0, t1) in groups:
                        gch = int(coff[t1] - coff[t0])
                        ohb = gat.tile([128, gch, ohw], BF16, tag="ohb")
                        nc.sync.dma_start(
                            out=ohb[:],
                            in_=oh_dram[:, int(coff[t0]) * ohw:
                                        int(coff[t1]) * ohw])
                        for t in range(t0, t1):
                            base = int(coff[t] - coff[t0])
                            n_ch = int(cpt[t])
                            s_ps = psA.tile([128, 136], F32, tag="pacc",
                                            space="PSUM")
                            for j in range(n_ch):
                                nc.tensor.matmul(
                                    s_ps[:kd, :128],
                                    g_sb[:, int(coff[t]) + j, :],
                                    ohb[:, base + j, :128],
                                    start=(j == 0), stop=(j == n_ch - 1))
                            s_sb = stp.tile([kd, 128], BF16,
                                            tag=f"ssb{kd}")
                            nc.vector.tensor_copy(out=s_sb[:],
                                                  in_=s_ps[:kd, :128])
                            acc = psA.tile([128, 136], F32, tag="pacc",
                                           space="PSUM")
                            nc.tensor.matmul(acc[:, :128], wf_sb[:], s_sb[:],
                                             start=True, stop=True)
                            h_ap = h_sb[:, t * 128:(t + 1) * 128]
                            nc.scalar.activation(
                                out=h_ap, in_=acc[:, :128],
                                func=mybir.ActivationFunctionType.Relu,
                                bias=b_sb[:, 0:1])

                def gcn_layer2(src_dram, idx_sb, oh_dram, ohw, cpt, coff,
                               groups, h_sb, b_sb, gsum=None, qoff=0):
                    """Layer 2: bf16 dma_gather of h1@W2 rows + one-hot
                    scatter matmuls."""
                    for gi, (t0, t1) in enumerate(groups):
                        gch = int(coff[t1] - coff[t0])
                        e0 = int(coff[t0]) * 128
                        m = gatm.tile([128, gch, D], BF16, tag="mbufb")
                        nc.gpsimd.dma_gather(
                            m[:], src_dram[:],
                            idx_sb[:, e0 // 16:(e0 + gch * 128) // 16],
                            gch * 128, gch * 128, D, single_packet=False,
                            queue_num=(qoff + gi) % 4)
                        ohb = gat.tile([128, gch, ohw], BF16, tag="ohb2")
                        nc.sync.dma_start(
                            out=ohb[:],
                            in_=oh_dram[:, int(coff[t0]) * ohw:
                                        int(coff[t1]) * ohw])
                        for t in range(t0, t1):
                            nw = ohw if gsum is not None else 128
                            acc = psA.tile([128, 136], F32, tag="pacc",
                                           space="PSUM")
                            base = int(coff[t] - coff[t0])
                            n_ch = int(cpt[t])
                            for j in range(n_ch):
                                nc.tensor.matmul(
                                    acc[:, :nw], m[:, base + j, :],
                                    ohb[:, base + j, :nw],
                                    start=(j == 0), stop=(j == n_ch - 1))
                            h_ap = h_sb[:, t * 128:(t + 1) * 128]
                            nc.scalar.activation(
                                out=h_ap,
                                in_=acc[:, :128],
                                func=mybir.ActivationFunctionType.Identity,
                                bias=b_sb[:, 0:1])
                            if gsum is not None:
                                nc.vector.tensor_tensor(
                                    out=gsum[:], in0=gsum[:],
                                    in1=acc[:, 128:136], op=OP.add)

                def xw2_own(h_sb, w_sb, cci, ntile, nrows):
                    off = 0
                    while off < ntile:
                        nb = min(4, ntile - off)
                        acc = psB.tile([128, nb * 128], F32, tag="b512",
                                       space="PSUM")
                        for j in range(nb):
                            i = off + j
                            nc.tensor.matmul(
                                acc[:, j * 128:(j + 1) * 128],
                                h_sb[:, i * 128:(i + 1) * 128],
                                w_sb[:], start=True, stop=True)
                        stg = stp.tile([128, nb, 128], BF16, tag="stg")
                        nc.vector.tensor_copy(out=stg[:], in_=acc[:])
                        nfull = min(nb, (nrows - off * 128) // 128)
                        if nfull > 0:
                            nc.sync.dma_start(
                                out=cci[off * 128:(off + nfull) * 128, :]
                                .rearrange("(t p) f -> p t f", p=128),
                                in_=stg[:, :nfull, :])
                        rem = nrows - (off + nfull) * 128
                        if 0 < rem < 128 and nfull < nb:
                            nc.sync.dma_start(
                                out=cci[(off + nfull) * 128:nrows, :],
                                in_=stg[:rem, nfull, :])
                        off += nb

                # ---- p layer 1 (no gather)
                h1p = bigp.tile([128, PPAD], BF16, tag="bigh")
                gcn_layer1(gp_sb, KP, ohp_d, 136, cpt_p, coff_p, grp_p,
                           h1p, b1p_sb, wf1p_sb)
                # ---- XW2p own rows + AllGather (bf16)
                xw2_own(h1p, w2p_sb, ccip, NPT, PPAD)
                nc.gpsimd.collective_compute(
                    "AllGather", OP.bypass,
                    ins=[ccip[:]], outs=[ccop[:]],
                    replica_groups=[list(range(NC))])

                # ---- v layer 1 (overlaps p AllGather)
                h1v = vsml.tile([128, VPAD], BF16, tag="h1v")
                gcn_layer1(gv_sb, KV, ohv_d, 128, cpt_v, coff_v, grp_v,
                           h1v, b1v_sb, wf1v_sb)
                xw2_own(h1v, w2v_sb, cciv, NVT, NVC)
                nc.gpsimd.collective_compute(
                    "AllGather", OP.bypass,
                    ins=[cciv[:]], outs=[ccov[:]],
                    replica_groups=[list(range(NC))])

                # ---- p layer 2 (with fused graph pooling); v layer 2 is
                # interleaved mid-chain so its pools finish under p's drain
                gsum = vsml.tile([128, GPC], F32, tag="gsum")
                nc.vector.memset(gsum[:], 0.0)
                h2p = bigp.tile([128, PPAD], BF16, tag="bigh")
                ph1 = grp_p[:len(grp_p) // 2]
                ph2 = grp_p[len(grp_p) // 2:]
                gcn_layer2(ccop, idxp2_sb, ohp_d, 136, cpt_p, coff_p,
                           ph1, h2p, b2p_sb, gsum=gsum)
                h2v = vsml.tile([128, VPAD], F32, tag="h2v")
                gcn_layer2(ccov, idxv1_sb, ohv_d, 128, cpt_v, coff_v,
                           grp_v, h2v, b2v_sb, qoff=len(ph1))
                # ---- v pools / v summed / cur_v
                gv = vsml.tile([128, GPC], F32, tag="gv")
                for g in range(GPC):
                    nc.vector.reduce_sum(out=gv[:, g:g + 1],
                                         in_=h2v[:, g * NVG:(g + 1) * NVG],
                                         axis=AX.X)
                gvs = vsml.tile([128, GPC], F32, tag="gvs")
                nc.vector.tensor_scalar(out=gvs[:], in0=gv[:],
                                        scalar1=1.0 / NVG, scalar2=None,
                                        op0=OP.mult)
                vsum = vsml.tile([128, VPAD], F32, tag="vsum")
                nc.vector.tensor_tensor(out=vsum[:], in0=h2v[:], in1=initv[:],
                                        op=OP.add)
                for g in range(GPC):
                    nc.scalar.activation(
                        out=vsum[:, g * NVG:(g + 1) * NVG],
                        in_=vsum[:, g * NVG:(g + 1) * NVG],
                        func=mybir.ActivationFunctionType.Identity,
                        bias=gvs[:, g:g + 1])
                # cur_v = vsum rows selected by action, via transpose + matmul
                curv_ps = psA.tile([128, GPC], F32, tag="pacc", space="PSUM")
                for k in range(NVT):
                    trp = psA.tile([128, 128], F32, tag="pacc", space="PSUM")
                    nc.tensor.transpose(trp[:], vsum[:, k * 128:(k + 1) * 128],
                                        ident[:])
                    vs_rm = vsml.tile([128, 128], F32, tag="vsrm")
                    nc.vector.tensor_copy(out=vs_rm[:], in_=trp[:])
                    nc.tensor.matmul(curv_ps[:], vs_rm[:],
                                     sel_sb[:, k * GPC:(k + 1) * GPC],
                                     start=(k == 0), stop=(k == NVT - 1))
                curv_sb = vsml.tile([128, GPC], F32, tag="curvsb")
                nc.vector.tensor_copy(out=curv_sb[:], in_=curv_ps[:])
                gcn_layer2(ccop, idxp2_sb, ohp_d, 136, cpt_p, coff_p,
                           ph2, h2p, b2p_sb, gsum=gsum,
                           qoff=len(ph1) + len(grp_v))

                # gc = gsum/NPG + b2p + curv   (graph mean of h2 + cur_v)
                gc = vsml.tile([128, GPC], F32, tag="gc")
                nc.vector.tensor_scalar(out=gc[:], in0=gsum[:],
                                        scalar1=1.0 / NPG,
                                        scalar2=b2p_sb[:, 0:1],
                                        op0=OP.mult, op1=OP.add)
                nc.vector.tensor_tensor(out=gc[:], in0=gc[:], in1=curv_sb[:],
                                        op=OP.add)
                # broadcast gc per graph across permuted columns via PE
                gcT_ps = psA.tile([128, 128], F32, tag="pacc", space="PSUM")
                nc.tensor.transpose(gcT_ps[:GPC, :],
                                    gc[:], ident[:])
                gcT = vsml.tile([GPC, 128], F32, tag="gcT")
                nc.vector.tensor_copy(out=gcT[:], in_=gcT_ps[:GPC, :])

                state = bigp.tile([128, PPAD], BF16, tag="bigh")
                nc.vector.tensor_tensor(out=state[:], in0=h2p[:],
                                        in1=initp[:], op=OP.add)
                for n in range(PPAD // 512):
                    sl = slice(n * 512, (n + 1) * 512)
                    gcx = psB.tile([128, 512], F32, tag="b512", space="PSUM")
                    nc.tensor.matmul(gcx[:], gcT[:], gt_sb[:, sl],
                                     start=True, stop=True)
                    nc.vector.tensor_tensor(out=state[:, sl],
                                            in0=state[:, sl],
                                            in1=gcx[:], op=OP.add)

                # ---- MLP
                mh1a = bigp.tile([128, PPAD], BF16, tag="bigh")
                mh1b = bigp.tile([128, PPAD], BF16, tag="bigh")
                for (w_sb, b_sb, mh) in ((w1a_sb, b1a_sb, mh1a),
                                         (w1b_sb, b1b_sb, mh1b)):
                    for n in range(PPAD // 512):
                        sl = slice(n * 512, (n + 1) * 512)
                        acc = psB.tile([128, 512], F32, tag="b512",
                                       space="PSUM")
                        nc.tensor.matmul(acc[:], w_sb[:], state[:, sl],
                                         start=True, stop=True)
                        nc.scalar.activation(
                            out=mh[:, sl], in_=acc[:],
                            func=mybir.ActivationFunctionType.Relu,
                            bias=b_sb[:, 0:1])
                mh2 = bigp.tile([128, PPAD], BF16, tag="bigh")
                for n in range(PPAD // 512):
                    sl = slice(n * 512, (n + 1) * 512)
                    acc = psB.tile([128, 512], F32, tag="b512", space="PSUM")
                    nc.tensor.matmul(acc[:], w2a_sb[:], mh1a[:, sl],
                                     start=True, stop=False)
                    nc.tensor.matmul(acc[:], w2b_sb[:], mh1b[:, sl],
                                     start=False, stop=True)
                    nc.scalar.activation(
                        out=mh2[:, sl], in_=acc[:],
                        func=mybir.ActivationFunctionType.Relu,
                        bias=b2m_sb[:, 0:1])
                for n in range(PPAD // 512):
                    sl = slice(n * 512, (n + 1) * 512)
                    accl = psA.tile([1, 512], F32, tag="pacc", space="PSUM")
                    nc.tensor.matmul(accl[:], w3_sb[:], mh2[:, sl],
                                     start=True, stop=True)
                    lgc = vsml.tile([1, 512], F32, tag="lgc")
                    nc.vector.tensor_scalar(
                        out=lgc[0:1, :], in0=accl[:], scalar1=b3_sb[0:1, 0:1],
                        scalar2=None, op0=OP.add)
                    nc.sync.dma_start(out=out_d[0:1, sl], in_=lgc[0:1, :])

    nc.compile()
    return nc


# ------------------------------------------------------------------- kernel

def kernel(**inputs):
    global LAST_EXEC_NS
    f = lambda k: np.asarray(inputs[k], dtype=np.float32)

    # edge preprocessing
    sp, dlp, nmp, grp_, cpt_p, perm = _prep_edges_balanced(
        np.asarray(inputs["p_edge_index"]), NP, NPC, NPT, PPAD)
    sv, dlv, nmv, cpt_v = _prep_edges(np.asarray(inputs["v_edge_index"]),
                                      NV, NVC, NVT)
    # L2 p rows live at permuted positions: node -> core*PPAD + tile*128+pos
    posmap = np.empty(NP, np.int64)          # node -> tile*128+pos
    for c in range(NC):
        valid = perm[c] >= 0
        posmap[perm[c][valid]] = np.nonzero(valid)[0]
    sp2 = (sp // NPC) * PPAD + posmap[sp]

    idxs_p2 = _pack_idx(sp2)
    idxs_v1 = _pack_idx(sv)
    oh_p = _build_onehots(dlp, nmp, grp_)
    oh_v = _build_onehots(dlv, nmv)

    # weights
    p_x = f("p_x"); v_x = f("v_x")
    wep = np.vstack([f("p_lin_w"), f("p_lin_b")[None, :]])
    wev = np.vstack([f("v_lin_w"), f("v_lin_b")[None, :]])
    wf1p = wep @ f("p_gcn_w1")
    wf1v = wev @ f("v_gcn_w1")
    pxT = np.vstack([p_x.T, np.ones((1, NP), np.float32)])
    vxT = np.vstack([v_x.T, np.ones((1, NV), np.float32)])
    act = np.asarray(inputs["high_level_action"]).astype(np.int64)

    # pre-gathered per-edge raw features (layer 1 needs no device gather)
    Gp = _build_G(sp, pxT.astype(NPBF))
    Gv = _build_G(sv, vxT.astype(NPBF))

    base = {
        "wf1p": wf1p.astype(NPBF), "wep": wep,
        "w2p": f("p_gcn_w2").astype(NPBF),
        "b1p": f("p_gcn_b1")[:, None], "b2p": f("p_gcn_b2")[:, None],
        "wf1v": wf1v.astype(NPBF), "wev": wev,
        "w2v": f("v_gcn_w2").astype(NPBF),
        "b1v": f("v_gcn_b1")[:, None], "b2v": f("v_gcn_b2")[:, None],
        "w1a": f("low_w1")[:, :D].astype(NPBF),
        "w1b": f("low_w1")[:, D:].astype(NPBF),
        "b1a": f("low_b1")[:D, None], "b1b": f("low_b1")[D:, None],
        "w2a": f("low_w2")[:D, :].astype(NPBF),
        "w2b": f("low_w2")[D:, :].astype(NPBF),
        "b2m": f("low_b2")[:, None],
        "w3": f("low_w3").astype(NPBF), "b3": f("low_b3")[:, None],
    }
    base = {k: (np.ascontiguousarray(v) if v.dtype == NPBF
                else np.ascontiguousarray(v, dtype=np.float32))
            for k, v in base.items()}

    pgraph = np.asarray(inputs["p_batch"]).astype(np.int64)

    in_maps = []
    for c in range(NC):
        selm = np.zeros((128, NVT * GPC), np.float32)
        for g in range(GPC):
            r = g * NVG + int(act[c * GPC + g])
            selm[r % 128, (r // 128) * GPC + g] = 1.0
        # permuted own features (gap columns zero) + graph one-hot
        pxo = np.zeros((KP, PPAD), np.float32)
        gtm = np.zeros((GPC, PPAD), np.float32)
        valid = perm[c] >= 0
        cols = np.nonzero(valid)[0]
        nodes = perm[c][valid]
        pxo[:, cols] = pxT[:, nodes]
        gtm[pgraph[nodes] % GPC, cols] = 1.0
        m = dict(base)
        m["p_xT_own"] = pxo
        m["v_xT_own"] = np.ascontiguousarray(
            vxT[:, c * NVC:(c + 1) * NVC])
        m["sel"] = selm
        m["gt"] = gtm
        m["idxs_p2"] = idxs_p2[c]
        m["idxs_v1"] = idxs_v1[c]
        m["oh_p"] = oh_p[c]; m["oh_v"] = oh_v[c]
        m["Gp"] = Gp[c]; m["Gv"] = Gv[c]
        in_maps.append(m)

    nc = _build(cpt_p, cpt_v)
    res = run_bass_kernel_spmd(nc, in_maps, core_ids=list(range(NC)),
                               trace=TRACE)
    LAST_EXEC_NS = res.exec_time_ns
    out = np.empty((NC, NPC), np.float32)
    for c in range(NC):
        lgv = res.results[c]["out"][0]
        valid = perm[c] >= 0
        out[c][perm[c][valid] - c * NPC] = lgv[valid]
    return out.reshape(B, NPG).astype(np.float32)
